# revision 27
# baseline (speedup 1.0000x reference)
"""Causal GQA self-attention block (B=4, S=2048, D=2048, 16 q-heads / 4 kv-heads)
on 8 Trainium2 NeuronCores.

Sharding: TP2 x DP4. Core c handles batch b = c//2 and head-half h = c%2
(q-heads 8h..8h+7, kv-heads 2h..2h+1). Each core computes a [2048, 2048]
partial of the output projection (transposed, [out_dim, seq]); the host sums
the two TP partials per batch and transposes back.

Per-core pipeline (all matmuls bf16 inputs / fp32 PSUM accumulation):
  A. QKV projection (lhsT = xT tiles stationary, fused W^T moving), fused
     RMS-norm + RoPE + gain scaling in natural layout, then DMA-XBAR
     transposes of Q,K head tiles to [hd, seq]; V kept natural [seq, hd].
  B+C fused, looping qt (512-query slices) outer / head inner:
     scores S^T[k,q] blocks = K_blk^T Q^T with fully-masked column ranges
     trimmed, single [128,128] additive triangle mask on diagonal blocks,
     plain exp (rms-normed q,k bound |score| <= sqrt(hd)), bf16 softmax
     accumulator, denominator via ones-matmul partition reduction + PE
     broadcast (emitted one head late to avoid PE head-of-line stalls),
     unnormalized y^T in PSUM normalized at eviction. The output projection
     for qt-1's 512-column slice is interleaved two output tiles at a time
     between heads of qt, keeping the tensor engine dense and HAM-warm.
"""
import sys

if "/opt/trn_rl_repo" not in sys.path:
    sys.path.insert(0, "/opt/trn_rl_repo")

import numpy as np
import ml_dtypes

import concourse.bass as bass
import concourse.mybir as mybir
from concourse import bacc
from concourse.tile import TileContext
from concourse.bass_utils import run_bass_kernel_spmd
from concourse.masks import make_identity

BF16 = mybir.dt.bfloat16
F32 = mybir.dt.float32
AF = mybir.ActivationFunctionType
OP = mybir.AluOpType

DIM = 2048
SEQ = 2048
BATCH = 4
HD = 128
NH_L = 8            # q heads per core
NKV_L = 2           # kv heads per core
NHT = NH_L + NKV_L  # normed heads per seq tile
QKV = (NH_L + 2 * NKV_L) * HD   # 1536
N_ST = SEQ // 128   # 16 seq tiles
N_DC = DIM // 128   # 16 contraction chunks
N_QT = SEQ // 512   # 4 query tiles of 512
EPS = 1.1920928955078125e-07
NEG = -1.0e30
N_CORES = 8

_CACHED_NC = None


def _build_nc():
    nc = bacc.Bacc(
        "TRN2",
        target_bir_lowering=False,
        debug=False,
        num_devices=N_CORES,
    )
    xT = nc.dram_tensor("xT", [DIM, SEQ], BF16, kind="ExternalInput")
    wt = nc.dram_tensor("wt", [DIM, QKV], BF16, kind="ExternalInput")
    wpT = nc.dram_tensor("wpT", [NH_L * HD, DIM], BF16, kind="ExternalInput")
    cs = nc.dram_tensor("cs", [SEQ, HD], BF16, kind="ExternalInput")
    gains = nc.dram_tensor("gains", [128, NHT], F32, kind="ExternalInput")
    maskt = nc.dram_tensor("maskt", [128, 128], F32, kind="ExternalInput")
    out = nc.dram_tensor("out", [DIM, SEQ], F32, kind="ExternalOutput")

    with TileContext(nc) as tc, \
         nc.allow_low_precision(reason="bf16 softmax accumulate"):
        with tc.tile_pool(name="const", bufs=1) as const, \
             tc.tile_pool(name="persist", bufs=1) as persist:
            ident = const.tile([128, 128], BF16)
            make_identity(nc, ident[:])
            ones_mat = const.tile([128, 128], BF16)
            nc.gpsimd.memset(ones_mat[:], 1.0)
            eps_sb = const.tile([128, 1], F32)
            nc.gpsimd.memset(eps_sb[:], EPS)
            gains_sb = const.tile([128, NHT], F32)
            nc.sync.dma_start(gains_sb[:], gains[:])
            mask_sb = const.tile([128, 128], F32)
            nc.sync.dma_start(mask_sb[:], maskt[:])

            # persistent activations
            qt_sb = persist.tile([128, NH_L * SEQ], BF16)   # Q^T per head
            kt_sb = persist.tile([128, NKV_L * SEQ], BF16)  # K^T per kv head
            v_sb = persist.tile([128, N_ST * NKV_L * HD], BF16)  # V natural
            yt_sb = persist.tile([128, NH_L * SEQ], BF16)   # y^T per head

            # ---------------- stage A: QKV projection + norm/rope/transpose
            with tc.tile_pool(name="a_w", bufs=1) as a_w, \
                 tc.tile_pool(name="a_sbuf", bufs=2) as a_sbuf, \
                 tc.tile_pool(name="a_stat", bufs=2) as a_stat, \
                 tc.tile_pool(name="a_psum", bufs=2, space="PSUM") as a_psum, \
                 tc.tile_pool(name="t_psum", bufs=1, space="PSUM") as t_psum:
                wt_sb = a_w.tile([128, N_DC * QKV], BF16)

                def emit_transposes(st, natq):
                    # PE transposes into one PSUM tile, then two batched
                    # strided evictions on Act (q-heads, k-heads)
                    tp = t_psum.tile([128, NHT * 128], BF16, tag="tp")
                    for j in range(NHT):
                        nc.tensor.transpose(
                            tp[:, j * 128:(j + 1) * 128],
                            natq[:, j * 128:(j + 1) * 128], ident[:])
                    qdst = qt_sb[:].rearrange(
                        "p (j s) -> p j s", j=NH_L)[:, :, st * 128:(st + 1) * 128]
                    nc.scalar.copy(
                        qdst, tp[:, :NH_L * 128].rearrange(
                            "p (j s) -> p j s", j=NH_L))
                    kdst = kt_sb[:].rearrange(
                        "p (j s) -> p j s", j=NKV_L)[:, :, st * 128:(st + 1) * 128]
                    nc.scalar.copy(
                        kdst, tp[:, NH_L * 128:].rearrange(
                            "p (j s) -> p j s", j=NKV_L))

                prev = None
                xt_pair = None
                for st in range(N_ST):
                    if st % 2 == 0:
                        # load xT for two seq-tiles at once: 512B contiguous
                        # runs per partition instead of 256B, half the DMAs
                        xt_pair = a_sbuf.tile([128, N_DC * 256], BF16, tag="xt")
                        for c in range(N_DC):
                            nc.sync.dma_start(
                                xt_pair[:, c * 256:(c + 1) * 256],
                                xT[c * 128:(c + 1) * 128,
                                   st * 128:(st + 2) * 128])
                    half = st % 2
                    cs_t = a_sbuf.tile([128, HD], BF16, tag="cs")
                    nc.sync.dma_start(cs_t[:], cs[st * 128:(st + 1) * 128, :])
                    if st == 0:
                        # weight loads are emitted after st=0's xT/cs loads
                        # so the first QKV matmuls aren't queued behind 6MB
                        # of weight DMA
                        for c in range(N_DC):
                            nc.sync.dma_start(
                                wt_sb[:, c * QKV:(c + 1) * QKV],
                                wt[c * 128:(c + 1) * 128, :])

                    qkv_ps = a_psum.tile([128, QKV], F32, tag="qkv")
                    for c in range(N_DC):
                        for n in range(QKV // 512):
                            nc.tensor.matmul(
                                qkv_ps[:, n * 512:(n + 1) * 512],
                                xt_pair[:, c * 256 + half * 128:
                                        c * 256 + half * 128 + 128],
                                wt_sb[:, c * QKV + n * 512: c * QKV + (n + 1) * 512],
                                start=(c == 0), stop=(c == N_DC - 1))

                    if prev is not None:
                        emit_transposes(*prev)

                    # stage Q,K through a bf16 cast on Act so every rope DVE
                    # op runs in the 2x 16-bit mode; rms statistics from the
                    # bf16 copy (batched square on Act + strided DVE reduce)
                    qbf = a_sbuf.tile([128, NHT * 128], BF16, tag="qbf")
                    nc.scalar.copy(qbf[:], qkv_ps[:, :NHT * 128])
                    sq = a_stat.tile([128, NHT * 128], F32, tag="sq")
                    nc.scalar.activation(sq[:], qbf[:], AF.Square)
                    ssq = a_stat.tile([128, NHT], F32, tag="ssq")
                    nc.vector.tensor_reduce(
                        ssq[:], sq[:].rearrange("p (h s) -> p h s", h=NHT),
                        axis=mybir.AxisListType.X, op=OP.add)
                    rr = a_stat.tile([128, NHT], F32, tag="rr")
                    nc.scalar.activation(rr[:], ssq[:], AF.Sqrt,
                                         scale=1.0 / HD, bias=eps_sb[:])
                    ri = a_stat.tile([128, NHT], F32, tag="ri")
                    nc.vector.reciprocal(ri[:], rr[:])
                    rq = a_stat.tile([128, NHT], F32, tag="rq")
                    nc.vector.tensor_mul(rq[:], ri[:], gains_sb[:])

                    # batched rope (strided APs over all 10 heads), then
                    # per-head scale by rq in place — all bf16
                    natq = a_sbuf.tile([128, NHT * 128], BF16, tag="natq")
                    qv = qbf[:].rearrange(
                        "p (h two s) -> p h two s", h=NHT, two=2)
                    nv = natq[:].rearrange(
                        "p (h two s) -> p h two s", h=NHT, two=2)
                    u1 = qv[:, :, 0, :]
                    u2 = qv[:, :, 1, :]
                    o1 = nv[:, :, 0, :]
                    o2 = nv[:, :, 1, :]
                    cob = cs_t[:, 0:64].unsqueeze(1).broadcast_to((128, NHT, 64))
                    sib = cs_t[:, 64:128].unsqueeze(1).broadcast_to((128, NHT, 64))
                    ta = a_stat.tile([128, NHT * 64], BF16, tag="ta")
                    tb = a_stat.tile([128, NHT * 64], BF16, tag="tb")
                    tav = ta[:].rearrange("p (h s) -> p h s", h=NHT)
                    tbv = tb[:].rearrange("p (h s) -> p h s", h=NHT)
                    nc.vector.tensor_mul(tav, u1, cob)
                    nc.vector.tensor_mul(tbv, u2, sib)
                    nc.vector.tensor_add(o1, tav, tbv)
                    nc.vector.tensor_mul(tav, u2, cob)
                    nc.vector.tensor_mul(tbv, u1, sib)
                    nc.vector.tensor_sub(o2, tav, tbv)
                    for j in range(NHT):
                        nc.vector.tensor_scalar_mul(
                            natq[:, j * 128:(j + 1) * 128],
                            natq[:, j * 128:(j + 1) * 128], rq[:, j:j + 1])

                    # V eviction (natural layout, bf16) on Act
                    nc.scalar.copy(
                        v_sb[:, st * NKV_L * HD:(st + 1) * NKV_L * HD],
                        qkv_ps[:, NHT * 128:])

                    prev = (st, natq)
                emit_transposes(*prev)

            # ---------------- stage B+C fused: attention + output projection
            with tc.tile_pool(name="b_w", bufs=1) as b_w, \
                 tc.tile_pool(name="b_sbuf", bufs=3) as b_sbuf, \
                 tc.tile_pool(name="b_acc", bufs=2) as b_acc, \
                 tc.tile_pool(name="o_sbuf", bufs=3) as o_sbuf, \
                 tc.tile_pool(name="s_psum", bufs=2, space="PSUM") as s_psum, \
                 tc.tile_pool(name="y_psum", bufs=2, space="PSUM") as y_psum, \
                 tc.tile_pool(name="w_psum", bufs=2, space="PSUM") as w_psum:
                wp_sb = b_w.tile([128, NH_L * DIM], BF16)
                for ic in range(NH_L):
                    nc.sync.dma_start(
                        wp_sb[:, ic * DIM:(ic + 1) * DIM],
                        wpT[ic * 128:(ic + 1) * 128, :])

                def emit_attn_head(j, qt):
                    """scores + exp + bf16 accumulate + unnormalized y^T.
                    Returns (j, qt, acc, y_ps) for the deferred denominator."""
                    jj = j // (NH_L // NKV_L)
                    nblk = 4 * qt + 4
                    npair = nblk // 2
                    q0 = j * SEQ + qt * 512
                    acc = b_acc.tile([128, 512], BF16, tag="acc")
                    y_ps = y_psum.tile([128, 512], F32, tag="y")

                    def emit_y(pair):
                        g0, pp = pair
                        for half in range(2):
                            kb = 2 * g0 + half
                            d = kb - 4 * qt
                            s0 = 128 * d if d > 0 else 0
                            nc.tensor.matmul(
                                y_ps[:, s0:],
                                v_sb[:, kb * NKV_L * HD + jj * HD:
                                     kb * NKV_L * HD + (jj + 1) * HD],
                                pp[:, half * 512 + s0:(half + 1) * 512],
                                start=(kb == 0), stop=(kb == nblk - 1))

                    pend = None
                    for g in range(npair):
                        s_ps = s_psum.tile([128, 1024], F32, tag="s")
                        for half in range(2):
                            kb = 2 * g + half
                            d = kb - 4 * qt
                            s0 = 128 * d if d > 0 else 0
                            nc.tensor.matmul(
                                s_ps[:, half * 512 + s0:(half + 1) * 512],
                                kt_sb[:, jj * SEQ + kb * 128:
                                      jj * SEQ + (kb + 1) * 128],
                                qt_sb[:, q0 + s0:q0 + 512],
                                start=True, stop=True)
                            if d >= 0:
                                # triangle mask on the 128 partially-masked
                                # columns of the diagonal block
                                nc.vector.tensor_add(
                                    s_ps[:, half * 512 + s0:
                                         half * 512 + s0 + 128],
                                    s_ps[:, half * 512 + s0:
                                         half * 512 + s0 + 128],
                                    mask_sb[:])
                        # exp of the full pair; trimmed columns hold stale
                        # data and are never consumed downstream
                        p_bf = b_sbuf.tile([128, 1024], BF16, tag="p")
                        nc.scalar.activation(p_bf[:], s_ps[:], AF.Exp)
                        for half in range(2):
                            kb = 2 * g + half
                            d = kb - 4 * qt
                            s0 = 128 * d if d > 0 else 0
                            src = p_bf[:, half * 512 + s0:(half + 1) * 512]
                            if g == 0 and half == 0:
                                nc.vector.tensor_copy(acc[:], p_bf[:, :512])
                            else:
                                nc.vector.tensor_add(
                                    acc[:, s0:], acc[:, s0:], src)
                        if pend is not None:
                            emit_y(pend)
                        pend = (g, p_bf)
                    emit_y(pend)
                    return (j, qt, acc, y_ps)

                def emit_denominator(pend_den):
                    """one all-ones stationary matmul computes per-q-column
                    sums already broadcast to all 128 partitions; reciprocal
                    on DVE feeds the normalized eviction directly."""
                    j, qt, acc, y_ps = pend_den
                    q0 = j * SEQ + qt * 512
                    rb_ps = w_psum.tile([128, 512], F32, tag="w")
                    nc.tensor.matmul(
                        rb_ps[:], ones_mat[:], acc[:], start=True, stop=True)
                    rinv = b_sbuf.tile([128, 512], F32, tag="rinv")
                    nc.vector.reciprocal(rinv[:], rb_ps[:])
                    nc.vector.tensor_mul(
                        yt_sb[:, q0:q0 + 512], y_ps[:], rinv[:])

                def emit_proj(qt, ots):
                    """output projection for columns qt*512..+512, given ots."""
                    for ot in ots:
                        po_ps = w_psum.tile([128, 512], F32, tag="w")
                        for ic in range(NH_L):
                            nc.tensor.matmul(
                                po_ps[:],
                                wp_sb[:, ic * DIM + ot * 128:
                                      ic * DIM + (ot + 1) * 128],
                                yt_sb[:, ic * SEQ + qt * 512:
                                      ic * SEQ + (qt + 1) * 512],
                                start=(ic == 0), stop=(ic == NH_L - 1))
                        o_sb = o_sbuf.tile([128, 512], F32, tag="osb")
                        nc.scalar.copy(o_sb[:], po_ps[:])
                        nc.sync.dma_start(
                            out[ot * 128:(ot + 1) * 128,
                                qt * 512:(qt + 1) * 512], o_sb[:])

                pend_den = None
                for qt in range(N_QT):
                    for j in range(NH_L):
                        nxt = emit_attn_head(j, qt)
                        if pend_den is not None:
                            emit_denominator(pend_den)
                        pend_den = nxt
                        if qt > 0:
                            emit_proj(qt - 1, [2 * j, 2 * j + 1])
                emit_denominator(pend_den)
                emit_proj(N_QT - 1, list(range(16)))

    nc.compile()
    return nc


def _get_nc():
    global _CACHED_NC
    if _CACHED_NC is None:
        _CACHED_NC = _build_nc()
    return _CACHED_NC


def _make_rope_tables():
    inv_freq = 1.0 / (10000.0 ** (np.arange(0, HD, 2, dtype=np.float32) / HD))
    t = np.arange(SEQ, dtype=np.float32)
    freqs = np.outer(t, inv_freq)
    return np.concatenate(
        [np.cos(freqs), np.sin(freqs)], axis=1).astype(ml_dtypes.bfloat16)


def _make_maskt():
    col = np.arange(128)[None, :]
    row = np.arange(128)[:, None]
    return np.ascontiguousarray(
        np.where(row > col, NEG, 0.0).astype(np.float32))


def _prep_in_maps(x, Wq, Wk, Wv, Wproj, q_gain):
    x = np.asarray(x, dtype=np.float32)
    Wq = np.asarray(Wq, dtype=np.float32)
    Wk = np.asarray(Wk, dtype=np.float32)
    Wv = np.asarray(Wv, dtype=np.float32)
    Wproj = np.asarray(Wproj, dtype=np.float32)
    q_gain = np.asarray(q_gain, dtype=np.float32)

    bf = ml_dtypes.bfloat16
    cs = _make_rope_tables()
    maskt = _make_maskt()
    xT = [np.ascontiguousarray(x[b].T).astype(bf) for b in range(BATCH)]
    wt_h, wp_h, g_h = [], [], []
    for h in range(2):
        w = np.concatenate([
            Wq[1024 * h:1024 * (h + 1)],
            Wk[256 * h:256 * (h + 1)],
            Wv[256 * h:256 * (h + 1)]], axis=0)
        wt_h.append(np.ascontiguousarray(w.T).astype(bf))
        wp_h.append(np.ascontiguousarray(
            Wproj[:, 1024 * h:1024 * (h + 1)].T).astype(bf))
        g = np.concatenate([
            q_gain[8 * h:8 * (h + 1)] / np.sqrt(HD),
            np.ones(NKV_L, np.float32)]).astype(np.float32)
        g_h.append(np.ascontiguousarray(
            np.broadcast_to(g[None, :], (128, NHT))))

    in_maps = []
    for c in range(N_CORES):
        b, h = c // 2, c % 2
        in_maps.append({
            "xT": xT[b], "wt": wt_h[h], "wpT": wp_h[h], "cs": cs,
            "gains": g_h[h], "maskt": maskt,
        })
    return in_maps


def kernel(x, Wq, Wk, Wv, Wproj, q_gain):
    in_maps = _prep_in_maps(x, Wq, Wk, Wv, Wproj, q_gain)
    nc = _get_nc()
    res = run_bass_kernel_spmd(nc, in_maps, list(range(N_CORES))).results

    out = np.empty((BATCH, SEQ, DIM), dtype=np.float32)
    for b in range(BATCH):
        out[b] = (res[2 * b]["out"] + res[2 * b + 1]["out"]).T
    return out


# revision 31
# speedup vs baseline: 1.0594x; 1.0594x over previous
"""Causal GQA self-attention block (B=4, S=2048, D=2048, 16 q-heads / 4 kv-heads)
on 8 Trainium2 NeuronCores.

Sharding: TP2 x DP4. Core c handles batch b = c//2 and head-half h = c%2
(q-heads 8h..8h+7, kv-heads 2h..2h+1). Each core computes a [2048, 2048]
partial of the output projection (transposed, [out_dim, seq]); the host sums
the two TP partials per batch and transposes back.

Per-core pipeline (all matmuls bf16 inputs / fp32 PSUM accumulation):
  A. QKV projection (lhsT = xT tiles stationary, fused W^T moving), fused
     RMS-norm + RoPE + gain scaling in natural layout, then DMA-XBAR
     transposes of Q,K head tiles to [hd, seq]; V kept natural [seq, hd].
  B+C fused, looping qt (512-query slices) outer / head inner:
     scores S^T[k,q] blocks = K_blk^T Q^T with fully-masked column ranges
     trimmed, single [128,128] additive triangle mask on diagonal blocks,
     plain exp (rms-normed q,k bound |score| <= sqrt(hd)), bf16 softmax
     accumulator, denominator via ones-matmul partition reduction + PE
     broadcast (emitted one head late to avoid PE head-of-line stalls),
     unnormalized y^T in PSUM normalized at eviction. The output projection
     for qt-1's 512-column slice is interleaved two output tiles at a time
     between heads of qt, keeping the tensor engine dense and HAM-warm.
"""
import sys

if "/opt/trn_rl_repo" not in sys.path:
    sys.path.insert(0, "/opt/trn_rl_repo")

import numpy as np
import ml_dtypes

import concourse.bass as bass
import concourse.mybir as mybir
from concourse import bacc
from concourse.tile import TileContext
from concourse.bass_utils import run_bass_kernel_spmd
from concourse.masks import make_identity

BF16 = mybir.dt.bfloat16
F32 = mybir.dt.float32
AF = mybir.ActivationFunctionType
OP = mybir.AluOpType

DIM = 2048
SEQ = 2048
BATCH = 4
HD = 128
NH_L = 8            # q heads per core
NKV_L = 2           # kv heads per core
NHT = NH_L + NKV_L  # normed heads per seq tile
QKV = (NH_L + 2 * NKV_L) * HD   # 1536
N_ST = SEQ // 128   # 16 seq tiles
N_DC = DIM // 128   # 16 contraction chunks
N_QT = SEQ // 512   # 4 query tiles of 512
EPS = 1.1920928955078125e-07
NEG = -1.0e30
N_CORES = 8

_CACHED_NC = None


def _build_nc():
    nc = bacc.Bacc(
        "TRN2",
        target_bir_lowering=False,
        debug=False,
        num_devices=N_CORES,
    )
    xT = nc.dram_tensor("xT", [DIM, SEQ], BF16, kind="ExternalInput")
    wt = nc.dram_tensor("wt", [DIM, QKV], BF16, kind="ExternalInput")
    wpT = nc.dram_tensor("wpT", [NH_L * HD, DIM], BF16, kind="ExternalInput")
    cs = nc.dram_tensor("cs", [SEQ, HD], BF16, kind="ExternalInput")
    gains = nc.dram_tensor("gains", [128, NHT], F32, kind="ExternalInput")
    maskt = nc.dram_tensor("maskt", [128, 128], F32, kind="ExternalInput")
    out = nc.dram_tensor("out", [DIM, SEQ], F32, kind="ExternalOutput")

    with TileContext(nc) as tc, \
         nc.allow_low_precision(reason="bf16 softmax accumulate"):
        with tc.tile_pool(name="const", bufs=1) as const, \
             tc.tile_pool(name="persist", bufs=1) as persist:
            ident = const.tile([128, 128], BF16)
            make_identity(nc, ident[:])
            ident_f = const.tile([128, 128], F32)
            make_identity(nc, ident_f[:])
            ones_col = const.tile([128, 1], BF16)
            nc.gpsimd.memset(ones_col[:], 1.0)
            ones_row = const.tile([1, 128], F32)
            nc.gpsimd.memset(ones_row[:], 1.0)
            eps_sb = const.tile([128, 1], F32)
            nc.gpsimd.memset(eps_sb[:], EPS)
            gains_sb = const.tile([128, NHT], F32)
            nc.sync.dma_start(gains_sb[:], gains[:])
            mask_sb = const.tile([128, 128], F32)
            nc.sync.dma_start(mask_sb[:], maskt[:])

            # persistent activations
            qt_sb = persist.tile([128, NH_L * SEQ], BF16)   # Q^T per head
            kt_sb = persist.tile([128, NKV_L * SEQ], BF16)  # K^T per kv head
            v_sb = persist.tile([128, N_ST * NKV_L * HD], BF16)  # V natural
            yt_sb = persist.tile([128, NH_L * SEQ], BF16)   # y^T per head

            # ---------------- stage A: QKV projection + norm/rope/transpose
            with tc.tile_pool(name="a_w", bufs=1) as a_w, \
                 tc.tile_pool(name="a_sbuf", bufs=2) as a_sbuf, \
                 tc.tile_pool(name="a_stat", bufs=2) as a_stat, \
                 tc.tile_pool(name="a_psum", bufs=2, space="PSUM") as a_psum, \
                 tc.tile_pool(name="t_psum", bufs=1, space="PSUM") as t_psum:
                wt_sb = a_w.tile([128, N_DC * QKV], BF16)

                def emit_transposes(st, natq):
                    # PE transposes into one PSUM tile, then two batched
                    # strided evictions on Act (q-heads, k-heads)
                    tp = t_psum.tile([128, NHT * 128], BF16, tag="tp")
                    for j in range(NHT):
                        nc.tensor.transpose(
                            tp[:, j * 128:(j + 1) * 128],
                            natq[:, j * 128:(j + 1) * 128], ident[:])
                    qdst = qt_sb[:].rearrange(
                        "p (j s) -> p j s", j=NH_L)[:, :, st * 128:(st + 1) * 128]
                    nc.scalar.copy(
                        qdst, tp[:, :NH_L * 128].rearrange(
                            "p (j s) -> p j s", j=NH_L))
                    kdst = kt_sb[:].rearrange(
                        "p (j s) -> p j s", j=NKV_L)[:, :, st * 128:(st + 1) * 128]
                    nc.scalar.copy(
                        kdst, tp[:, NH_L * 128:].rearrange(
                            "p (j s) -> p j s", j=NKV_L))

                prev = None
                xt_pair = None
                for st in range(N_ST):
                    if st % 2 == 0:
                        # load xT for two seq-tiles in ONE strided-AP DMA
                        # (16 separate chunk DMAs pay ~565ns dispatch each
                        # and starve the first matmuls at kernel start)
                        xt_pair = a_sbuf.tile([128, N_DC * 256], BF16, tag="xt")
                        nc.sync.dma_start(
                            xt_pair[:].rearrange("p (c s) -> p c s", c=N_DC),
                            xT[:, st * 128:(st + 2) * 128].rearrange(
                                "(c p) s -> p c s", p=128))
                    half = st % 2
                    cs_t = a_sbuf.tile([128, HD], BF16, tag="cs")
                    nc.sync.dma_start(cs_t[:], cs[st * 128:(st + 1) * 128, :])
                    if st == 0:
                        # weight loads emitted after st=0's xT/cs loads, in
                        # 4-chunk strided-AP groups: few dispatches, but the
                        # first matmuls only wait on the first group
                        for g in range(4):
                            nc.sync.dma_start(
                                wt_sb[:, g * 4 * QKV:(g + 1) * 4 * QKV]
                                .rearrange("p (c q) -> p c q", c=4),
                                wt[g * 512:(g + 1) * 512, :]
                                .rearrange("(c p) q -> p c q", p=128))

                    qkv_ps = a_psum.tile([128, QKV], F32, tag="qkv")
                    for c in range(N_DC):
                        for n in range(QKV // 512):
                            nc.tensor.matmul(
                                qkv_ps[:, n * 512:(n + 1) * 512],
                                xt_pair[:, c * 256 + half * 128:
                                        c * 256 + half * 128 + 128],
                                wt_sb[:, c * QKV + n * 512: c * QKV + (n + 1) * 512],
                                start=(c == 0), stop=(c == N_DC - 1))

                    if prev is not None:
                        emit_transposes(*prev)

                    # stage Q,K through a bf16 cast on Act so every rope DVE
                    # op runs in the 2x 16-bit mode; rms statistics from the
                    # bf16 copy (batched square on Act + strided DVE reduce)
                    qbf = a_sbuf.tile([128, NHT * 128], BF16, tag="qbf")
                    nc.scalar.copy(qbf[:], qkv_ps[:, :NHT * 128])
                    sq = a_stat.tile([128, NHT * 128], F32, tag="sq")
                    nc.scalar.activation(sq[:], qbf[:], AF.Square)
                    ssq = a_stat.tile([128, NHT], F32, tag="ssq")
                    nc.vector.tensor_reduce(
                        ssq[:], sq[:].rearrange("p (h s) -> p h s", h=NHT),
                        axis=mybir.AxisListType.X, op=OP.add)
                    rr = a_stat.tile([128, NHT], F32, tag="rr")
                    nc.scalar.activation(rr[:], ssq[:], AF.Sqrt,
                                         scale=1.0 / HD, bias=eps_sb[:])
                    ri = a_stat.tile([128, NHT], F32, tag="ri")
                    nc.vector.reciprocal(ri[:], rr[:])
                    rq = a_stat.tile([128, NHT], F32, tag="rq")
                    nc.vector.tensor_mul(rq[:], ri[:], gains_sb[:])

                    # batched rope (strided APs over all 10 heads), then
                    # per-head scale by rq in place — all bf16
                    natq = a_sbuf.tile([128, NHT * 128], BF16, tag="natq")
                    qv = qbf[:].rearrange(
                        "p (h two s) -> p h two s", h=NHT, two=2)
                    nv = natq[:].rearrange(
                        "p (h two s) -> p h two s", h=NHT, two=2)
                    u1 = qv[:, :, 0, :]
                    u2 = qv[:, :, 1, :]
                    o1 = nv[:, :, 0, :]
                    o2 = nv[:, :, 1, :]
                    cob = cs_t[:, 0:64].unsqueeze(1).broadcast_to((128, NHT, 64))
                    sib = cs_t[:, 64:128].unsqueeze(1).broadcast_to((128, NHT, 64))
                    ta = a_stat.tile([128, NHT * 64], BF16, tag="ta")
                    tb = a_stat.tile([128, NHT * 64], BF16, tag="tb")
                    tav = ta[:].rearrange("p (h s) -> p h s", h=NHT)
                    tbv = tb[:].rearrange("p (h s) -> p h s", h=NHT)
                    nc.vector.tensor_mul(tav, u1, cob)
                    nc.vector.tensor_mul(tbv, u2, sib)
                    nc.vector.tensor_add(o1, tav, tbv)
                    nc.vector.tensor_mul(tav, u2, cob)
                    nc.vector.tensor_mul(tbv, u1, sib)
                    nc.vector.tensor_sub(o2, tav, tbv)
                    for j in range(NHT):
                        nc.vector.tensor_scalar_mul(
                            natq[:, j * 128:(j + 1) * 128],
                            natq[:, j * 128:(j + 1) * 128], rq[:, j:j + 1])

                    # V eviction (natural layout, bf16) on Act
                    nc.scalar.copy(
                        v_sb[:, st * NKV_L * HD:(st + 1) * NKV_L * HD],
                        qkv_ps[:, NHT * 128:])

                    prev = (st, natq)
                emit_transposes(*prev)

            # ---------------- stage B+C fused: attention + output projection
            with tc.tile_pool(name="b_w", bufs=1) as b_w, \
                 tc.tile_pool(name="b_sbuf", bufs=3) as b_sbuf, \
                 tc.tile_pool(name="b_acc", bufs=2) as b_acc, \
                 tc.tile_pool(name="o_sbuf", bufs=3) as o_sbuf, \
                 tc.tile_pool(name="s_psum", bufs=2, space="PSUM") as s_psum, \
                 tc.tile_pool(name="y_psum", bufs=2, space="PSUM") as y_psum, \
                 tc.tile_pool(name="w_psum", bufs=2, space="PSUM") as w_psum:
                wp_sb = b_w.tile([128, NH_L * DIM], BF16)
                nc.sync.dma_start(
                    wp_sb[:].rearrange("p (c q) -> p c q", c=NH_L),
                    wpT[:, :].rearrange("(c p) q -> p c q", p=128))

                def emit_attn_head(j, qt):
                    """scores + exp + bf16 accumulate + unnormalized y^T.
                    Returns (j, qt, acc, y_ps) for the deferred denominator."""
                    jj = j // (NH_L // NKV_L)
                    nblk = 4 * qt + 4
                    npair = nblk // 2
                    q0 = j * SEQ + qt * 512
                    acc = b_acc.tile([128, 512], BF16, tag="acc")
                    y_ps = y_psum.tile([128, 512], F32, tag="y")

                    def emit_y(pair):
                        g0, pp = pair
                        for half in range(2):
                            kb = 2 * g0 + half
                            d = kb - 4 * qt
                            s0 = 128 * d if d > 0 else 0
                            nc.tensor.matmul(
                                y_ps[:, s0:],
                                v_sb[:, kb * NKV_L * HD + jj * HD:
                                     kb * NKV_L * HD + (jj + 1) * HD],
                                pp[:, half * 512 + s0:(half + 1) * 512],
                                start=(kb == 0), stop=(kb == nblk - 1))

                    pend = None
                    for g in range(npair):
                        s_ps = s_psum.tile([128, 1024], F32, tag="s")
                        for half in range(2):
                            kb = 2 * g + half
                            d = kb - 4 * qt
                            s0 = 128 * d if d > 0 else 0
                            nc.tensor.matmul(
                                s_ps[:, half * 512 + s0:(half + 1) * 512],
                                kt_sb[:, jj * SEQ + kb * 128:
                                      jj * SEQ + (kb + 1) * 128],
                                qt_sb[:, q0 + s0:q0 + 512],
                                start=True, stop=True)
                            if d >= 0:
                                # triangle mask on the 128 partially-masked
                                # columns of the diagonal block
                                nc.vector.tensor_add(
                                    s_ps[:, half * 512 + s0:
                                         half * 512 + s0 + 128],
                                    s_ps[:, half * 512 + s0:
                                         half * 512 + s0 + 128],
                                    mask_sb[:])
                        # exp of the full pair; trimmed columns hold stale
                        # data and are never consumed downstream
                        p_bf = b_sbuf.tile([128, 1024], BF16, tag="p")
                        nc.scalar.activation(p_bf[:], s_ps[:], AF.Exp)
                        for half in range(2):
                            kb = 2 * g + half
                            d = kb - 4 * qt
                            s0 = 128 * d if d > 0 else 0
                            src = p_bf[:, half * 512 + s0:(half + 1) * 512]
                            if g == 0 and half == 0:
                                nc.vector.tensor_copy(acc[:], p_bf[:, :512])
                            else:
                                nc.vector.tensor_add(
                                    acc[:, s0:], acc[:, s0:], src)
                        if pend is not None:
                            emit_y(pend)
                        pend = (g, p_bf)
                    emit_y(pend)
                    return (j, qt, acc, y_ps)

                def emit_denominator(pend_den):
                    """per-q-column sums via transposed ones-matmuls (keeps
                    the reciprocal on 128 lanes), then PE broadcast back to
                    [128, 512] and normalized eviction to yt_sb."""
                    j, qt, acc, y_ps = pend_den
                    q0 = j * SEQ + qt * 512
                    dcol = w_psum.tile([128, 4], F32, tag="w")
                    for c2 in range(4):
                        nc.tensor.matmul(
                            dcol[:, c2:c2 + 1],
                            acc[:, c2 * 128:(c2 + 1) * 128],
                            ones_col[:], start=True, stop=True)
                    rinv = b_sbuf.tile([128, 4], F32, tag="rinv")
                    nc.vector.reciprocal(rinv[:], dcol[:])
                    rt_ps = w_psum.tile([1, 512], F32, tag="w")
                    for c2 in range(4):
                        nc.tensor.transpose(
                            rt_ps[:, c2 * 128:(c2 + 1) * 128],
                            rinv[:, c2:c2 + 1], ident_f[:])
                    rt_sb = b_sbuf.tile([1, 512], F32, tag="rts")
                    nc.vector.tensor_copy(rt_sb[:], rt_ps[:])
                    rb_ps = w_psum.tile([128, 512], F32, tag="w")
                    nc.tensor.matmul(
                        rb_ps[:], ones_row[:], rt_sb[:], start=True, stop=True)
                    rb_sb = b_sbuf.tile([128, 512], F32, tag="rbs")
                    nc.scalar.copy(rb_sb[:], rb_ps[:])
                    nc.vector.tensor_mul(
                        yt_sb[:, q0:q0 + 512], y_ps[:], rb_sb[:])

                def emit_proj(qt, ots):
                    """output projection for columns qt*512..+512, given ots."""
                    for ot in ots:
                        po_ps = w_psum.tile([128, 512], F32, tag="w")
                        for ic in range(NH_L):
                            nc.tensor.matmul(
                                po_ps[:],
                                wp_sb[:, ic * DIM + ot * 128:
                                      ic * DIM + (ot + 1) * 128],
                                yt_sb[:, ic * SEQ + qt * 512:
                                      ic * SEQ + (qt + 1) * 512],
                                start=(ic == 0), stop=(ic == NH_L - 1))
                        o_sb = o_sbuf.tile([128, 512], F32, tag="osb")
                        nc.scalar.copy(o_sb[:], po_ps[:])
                        nc.sync.dma_start(
                            out[ot * 128:(ot + 1) * 128,
                                qt * 512:(qt + 1) * 512], o_sb[:])

                pend_den = None
                for qt in range(N_QT):
                    for j in range(NH_L):
                        nxt = emit_attn_head(j, qt)
                        if pend_den is not None:
                            emit_denominator(pend_den)
                        pend_den = nxt
                        if qt > 0:
                            emit_proj(qt - 1, [2 * j, 2 * j + 1])
                emit_denominator(pend_den)
                emit_proj(N_QT - 1, list(range(16)))

    nc.compile()
    return nc


def _get_nc():
    global _CACHED_NC
    if _CACHED_NC is None:
        _CACHED_NC = _build_nc()
    return _CACHED_NC


def _make_rope_tables():
    inv_freq = 1.0 / (10000.0 ** (np.arange(0, HD, 2, dtype=np.float32) / HD))
    t = np.arange(SEQ, dtype=np.float32)
    freqs = np.outer(t, inv_freq)
    return np.concatenate(
        [np.cos(freqs), np.sin(freqs)], axis=1).astype(ml_dtypes.bfloat16)


def _make_maskt():
    col = np.arange(128)[None, :]
    row = np.arange(128)[:, None]
    return np.ascontiguousarray(
        np.where(row > col, NEG, 0.0).astype(np.float32))


def _prep_in_maps(x, Wq, Wk, Wv, Wproj, q_gain):
    x = np.asarray(x, dtype=np.float32)
    Wq = np.asarray(Wq, dtype=np.float32)
    Wk = np.asarray(Wk, dtype=np.float32)
    Wv = np.asarray(Wv, dtype=np.float32)
    Wproj = np.asarray(Wproj, dtype=np.float32)
    q_gain = np.asarray(q_gain, dtype=np.float32)

    bf = ml_dtypes.bfloat16
    cs = _make_rope_tables()
    maskt = _make_maskt()
    xT = [np.ascontiguousarray(x[b].T).astype(bf) for b in range(BATCH)]
    wt_h, wp_h, g_h = [], [], []
    for h in range(2):
        w = np.concatenate([
            Wq[1024 * h:1024 * (h + 1)],
            Wk[256 * h:256 * (h + 1)],
            Wv[256 * h:256 * (h + 1)]], axis=0)
        wt_h.append(np.ascontiguousarray(w.T).astype(bf))
        wp_h.append(np.ascontiguousarray(
            Wproj[:, 1024 * h:1024 * (h + 1)].T).astype(bf))
        g = np.concatenate([
            q_gain[8 * h:8 * (h + 1)] / np.sqrt(HD),
            np.ones(NKV_L, np.float32)]).astype(np.float32)
        g_h.append(np.ascontiguousarray(
            np.broadcast_to(g[None, :], (128, NHT))))

    in_maps = []
    for c in range(N_CORES):
        b, h = c // 2, c % 2
        in_maps.append({
            "xT": xT[b], "wt": wt_h[h], "wpT": wp_h[h], "cs": cs,
            "gains": g_h[h], "maskt": maskt,
        })
    return in_maps


def kernel(x, Wq, Wk, Wv, Wproj, q_gain):
    in_maps = _prep_in_maps(x, Wq, Wk, Wv, Wproj, q_gain)
    nc = _get_nc()
    res = run_bass_kernel_spmd(nc, in_maps, list(range(N_CORES))).results

    out = np.empty((BATCH, SEQ, DIM), dtype=np.float32)
    for b in range(BATCH):
        out[b] = (res[2 * b]["out"] + res[2 * b + 1]["out"]).T
    return out
